# revision 4
# baseline (speedup 1.0000x reference)
"""Trainium2 Bass kernel for nn_CNNFusing (session attention pooling).

Math (per session s of L=50 tokens, H=128):
  hidden = max(intra, inter)                                 [T, H]
  v_n[s] = hidden[last token of s]                           [B, H]
  y[t]   = W1 @ v_n[s(t)] + W2 @ hidden[t] + (b1 + b2)       [T, H]
  alpha[t] = q_w . sigmoid(y[t]) + q_b                       [T]
  s_g[s] = sum_{t in s} alpha[t] * hidden[t]                 [B, H]
  out[s] = [v_n[s], s_g[s]] @ W3.T + b3                      [B, H]

Strategy: shard sessions (contiguous token ranges) across 8 NeuronCores.
Per core, stream token slabs; compute max on GpSimd; PE-transpose hidden
to [H, t]; do the W2/W1 matmuls in f32r (TF32-ish, 1 cyc/row) with the
per-session v_n term injected via a stride-0 broadcast rhs; sigmoid on
ACT straight out of PSUM with the b1+b2 bias folded in; reduce against
q_w with a broadcast-stationary matmul that leaves alpha replicated on
all 128 partitions; fuse (+q_b) and the alpha*hidden product in one DVE
scalar_tensor_tensor; segment-sum with a strided tensor_reduce; finish
with small fp32 matmuls (exact for the v_n half of the output).
"""

import numpy as np

H = 128
L = 50
N_CORES = 8

_cache: dict = {}


def _numpy_ref(intra_item_emb, inter_item_emb, W1, b1, W2, b2, q_w, q_b, W3, b3,
               seq_len):
    hidden = np.maximum(intra_item_emb, inter_item_emb)
    nB = seq_len.shape[0]
    seg_ids = np.repeat(np.arange(nB), seq_len)
    last_idx = np.cumsum(seq_len) - 1
    v_n = hidden[last_idx]
    v_n_rep = v_n[seg_ids]
    z = v_n_rep @ W1.T + b1 + hidden @ W2.T + b2
    alpha = (1.0 / (1.0 + np.exp(-z))) @ q_w.T + q_b
    s_g = np.zeros((nB, hidden.shape[1]), np.float32)
    np.add.at(s_g, seg_ids, alpha * hidden)
    return (np.concatenate([v_n, s_g], axis=1) @ W3.T + b3).astype(np.float32)


def _build(t_core: int, q_b_val: float):
    """Build the per-core Bass program. t_core tokens (multiple of 3200)."""
    import concourse.bass as bass
    import concourse.mybir as mybir
    import concourse.tile as tile
    from concourse import bacc
    from concourse.masks import make_identity

    f32 = mybir.dt.float32
    f32r = mybir.dt.float32r

    MACRO = 3200            # tokens per macro-tile = 64 sessions = 25 chunks
    n_macro = t_core // MACRO
    assert t_core % MACRO == 0
    b_core = t_core // L    # sessions per core
    GRP = 400               # matmul group = 8 sessions, one PSUM bank
    n_grp = MACRO // GRP    # 8
    n_gblk = b_core // 128  # 128-session blocks for the final matmul

    nc = bacc.Bacc(trn_type="TRN2", num_devices=N_CORES)

    intra = nc.dram_tensor("intra", [t_core, H], f32, kind="ExternalInput").ap()
    inter = nc.dram_tensor("inter", [t_core, H], f32, kind="ExternalInput").ap()
    w1t_d = nc.dram_tensor("w1t", [H, H], f32r, kind="ExternalInput").ap()
    w2t_d = nc.dram_tensor("w2t", [H, H], f32r, kind="ExternalInput").ap()
    qwbc_d = nc.dram_tensor("qwbc", [H, H], f32r, kind="ExternalInput").ap()
    b12_d = nc.dram_tensor("b12", [H, 1], f32, kind="ExternalInput").ap()
    w3at_d = nc.dram_tensor("w3at", [H, H], f32, kind="ExternalInput").ap()
    w3bt_d = nc.dram_tensor("w3bt", [H, H], f32, kind="ExternalInput").ap()
    b3r_d = nc.dram_tensor("b3r", [1, H], f32, kind="ExternalInput").ap()
    out_d = nc.dram_tensor("h_s", [b_core, H], f32, kind="ExternalOutput").ap()

    with tile.TileContext(nc) as tc:
        with (
            tc.tile_pool(name="consts", bufs=1) as consts,
            tc.tile_pool(name="inp", bufs=2) as inp,
            tc.tile_pool(name="hid", bufs=2) as hid,
            tc.tile_pool(name="hts", bufs=2) as hts,
            tc.tile_pool(name="sig", bufs=2) as sig,
            tc.tile_pool(name="wts", bufs=2) as wts,
            tc.tile_pool(name="pers", bufs=1) as pers,
            tc.tile_pool(name="ps_t", bufs=2, space="PSUM") as ps_t,
            tc.tile_pool(name="ps_y", bufs=2, space="PSUM") as ps_y,
            tc.tile_pool(name="ps_a", bufs=2, space="PSUM") as ps_a,
        ):
            w1t = consts.tile([H, H], f32r)
            nc.sync.dma_start(w1t, w1t_d)
            w2t = consts.tile([H, H], f32r)
            nc.sync.dma_start(w2t, w2t_d)
            qwbc = consts.tile([H, H], f32r)
            nc.sync.dma_start(qwbc, qwbc_d)
            b12 = consts.tile([H, 1], f32)
            nc.sync.dma_start(b12, b12_d)
            w3at = consts.tile([H, H], f32)
            nc.sync.dma_start(w3at, w3at_d)
            w3bt = consts.tile([H, H], f32)
            nc.sync.dma_start(w3bt, w3bt_d)
            b3r = consts.tile([1, H], f32)
            nc.sync.dma_start(b3r, b3r_d)
            ident = consts.tile([H, H], f32)
            make_identity(nc, ident)
            ones1 = consts.tile([1, H], f32)
            nc.vector.memset(ones1, 1.0)

            s_gt = pers.tile([H, b_core], f32)   # [h, session]
            v_nt = pers.tile([H, b_core], f32r)   # [h, session]
            hs_sb = pers.tile([128, n_gblk, H], f32)

            intra_r = intra.rearrange("(m c p) h -> m p c h", p=128, c=25)
            inter_r = inter.rearrange("(m c p) h -> m p c h", p=128, c=25)

            for m in range(n_macro):
                ia = inp.tile([128, 25, H], f32, tag="ia")
                nc.sync.dma_start(ia, intra_r[m])
                ib = inp.tile([128, 25, H], f32, tag="ib")
                nc.sync.dma_start(ib, inter_r[m])

                # hidden = max(intra, inter), token-major [t, h]
                hd = hid.tile([128, 25, H], f32, tag="hd")
                nc.vector.tensor_tensor(hd, ia, ib, mybir.AluOpType.max)

                # transpose to [h, t]
                ht = hts.tile([H, MACRO], f32r, tag="ht")
                for tp in range(5):
                    pt = ps_t.tile([128, 640], f32, tag="pt")
                    for k in range(5):
                        nc.tensor.transpose(
                            pt[:, k * 128:(k + 1) * 128], hd[:, tp * 5 + k, :],
                            ident)
                    nc.scalar.copy(ht[:, tp * 640:(tp + 1) * 640], pt)

                # v_n columns: last token of each of the 64 sessions
                ht_sess = ht.rearrange("h (s l) -> h s l", l=L)
                nc.scalar.copy(v_nt[:, m * 64:(m + 1) * 64], ht_sess[:, :, L - 1])

                st = sig.tile([H, MACRO], f32r, tag="st")
                wt = wts.tile([H, MACRO], f32, tag="wt")
                for g in range(n_grp):
                    t0 = g * GRP
                    s0 = m * 64 + g * 8
                    py = ps_y.tile([128, GRP], f32, tag="py")
                    nc.tensor.matmul(py, lhsT=w2t,
                                     rhs=ht[:, t0:t0 + GRP],
                                     start=True, stop=False)
                    u_rhs = (v_nt[:, s0:s0 + 8][:, :, None]
                             .to_broadcast((H, 8, L)))
                    nc.tensor.matmul(py, lhsT=w1t, rhs=u_rhs,
                                     start=False, stop=True)
                    nc.scalar.activation(
                        st[:, t0:t0 + GRP], py,
                        mybir.ActivationFunctionType.Sigmoid, bias=b12)
                    pa = ps_a.tile([128, GRP], f32, tag="pa")
                    nc.tensor.matmul(pa, lhsT=qwbc,
                                     rhs=st[:, t0:t0 + GRP],
                                     start=True, stop=True)
                    # wt = (alpha_tilde + q_b) * hT
                    nc.vector.scalar_tensor_tensor(
                        out=wt[:, t0:t0 + GRP], in0=pa, scalar=float(q_b_val),
                        in1=ht[:, t0:t0 + GRP].bitcast(f32),
                        op0=mybir.AluOpType.add, op1=mybir.AluOpType.mult)

                # segment sum over each session's 50 tokens
                nc.vector.tensor_reduce(
                    s_gt[:, m * 64:(m + 1) * 64],
                    wt.rearrange("h (s l) -> h s l", l=L),
                    axis=mybir.AxisListType.X, op=mybir.AluOpType.add)

            # final: out[s, :] = v_n W3a^T + s_g W3b^T + b3   (fp32, exact)
            for gb in range(n_gblk):
                pf_full = ps_t.tile([128, 640], f32, tag="pt", name="pf")
                pf = pf_full[:, :H]
                nc.tensor.matmul(pf, lhsT=v_nt[:, gb * 128:(gb + 1) * 128].bitcast(f32),
                                 rhs=w3at, start=True, stop=False)
                nc.tensor.matmul(pf, lhsT=s_gt[:, gb * 128:(gb + 1) * 128],
                                 rhs=w3bt, start=False, stop=False)
                nc.tensor.matmul(pf, lhsT=ones1, rhs=b3r,
                                 start=False, stop=True)
                nc.vector.tensor_copy(hs_sb[:, gb, :], pf)

            nc.sync.dma_start(out_d.rearrange("(g p) h -> p g h", p=128), hs_sb)

    nc.compile()
    return nc


def kernel(intra_item_emb, inter_item_emb, W1, b1, W2, b2, q_w, q_b, W3, b3,
           seq_len):
    intra_item_emb = np.ascontiguousarray(np.asarray(intra_item_emb, np.float32))
    inter_item_emb = np.ascontiguousarray(np.asarray(inter_item_emb, np.float32))
    W1 = np.asarray(W1, np.float32)
    b1 = np.asarray(b1, np.float32)
    W2 = np.asarray(W2, np.float32)
    b2 = np.asarray(b2, np.float32)
    q_w = np.asarray(q_w, np.float32)
    q_b = np.asarray(q_b, np.float32)
    W3 = np.asarray(W3, np.float32)
    b3 = np.asarray(b3, np.float32)
    seq_len = np.asarray(seq_len)

    T, h = intra_item_emb.shape
    B = seq_len.shape[0]
    if (h != H or not np.all(seq_len == L) or T != B * L
            or T % (N_CORES * 3200) != 0):
        return _numpy_ref(intra_item_emb, inter_item_emb, W1, b1, W2, b2, q_w,
                          q_b, W3, b3, seq_len)

    from concourse.bass_utils import run_bass_kernel_spmd

    t_core = T // N_CORES
    key = (t_core, float(q_b[0]))
    if key not in _cache:
        _cache.clear()
        _cache[key] = _build(t_core, float(q_b[0]))
    nc = _cache[key]

    w1t = np.ascontiguousarray(W1.T)
    w2t = np.ascontiguousarray(W2.T)
    qwbc = np.ascontiguousarray(np.repeat(q_w.reshape(H, 1), H, axis=1))
    b12 = np.ascontiguousarray((b1 + b2).reshape(H, 1))
    w3at = np.ascontiguousarray(W3[:, :H].T)
    w3bt = np.ascontiguousarray(W3[:, H:].T)
    b3r = np.ascontiguousarray(b3.reshape(1, H))

    in_maps = []
    for c in range(N_CORES):
        sl = slice(c * t_core, (c + 1) * t_core)
        in_maps.append({
            "intra": intra_item_emb[sl],
            "inter": inter_item_emb[sl],
            "w1t": w1t, "w2t": w2t, "qwbc": qwbc, "b12": b12,
            "w3at": w3at, "w3bt": w3bt, "b3r": b3r,
        })

    res = run_bass_kernel_spmd(nc, in_maps, core_ids=list(range(N_CORES)))
    return np.concatenate([res.results[c]["h_s"] for c in range(N_CORES)],
                          axis=0)
